# revision 1
# baseline (speedup 1.0000x reference)
"""RWKV5 block, sequence-parallel across 8 trn2 cores.

Core c -> batch c//2, sequence half c%2 (tokens t0 = half*1024, TL=1024
= 2 recurrence chunks of TC=512). Each core runs FULL-width GEMMs
(C=2048, DF=7168) on its token half; every weight is streamed from HBM
once (Wv twice). Cross-core traffic per pair: one 512KB state AllGather
(recurrent state after chunk 1 -> second half) plus an 8KB x' halo
column AllGather for the ChannelMix time-shift.

Layout: activations channel-major [C, T]. v kept time-major [T, C]
(VT) for the attention a@v and k^T@v contractions.
"""
import numpy as np
import concourse.bass as bass
import concourse.mybir as mybir
import concourse.tile as tile
from concourse import bacc
from concourse.masks import make_identity

f32 = mybir.dt.float32
bf16 = mybir.dt.bfloat16
AOT = mybir.AluOpType
AFT = mybir.ActivationFunctionType

C = 2048
H = 32         # heads
S = 64         # head dim
TC = 512       # recurrence chunk
TL = 1024      # local tokens per core
NCH = TL // TC # 2 local chunks
DF = 7168
P = 128
NK = C // P    # 16 channel chunks
NP = H // 2    # 16 head pairs
NJ = DF // P   # 56
NQ = 4         # DF quarters
JQ = NJ // NQ  # 14 j-chunks per quarter
EPS = 1e-5
HS_DIV = float(np.sqrt(S))
GROUPS = [[0, 1], [2, 3], [4, 5], [6, 7]]
TS = TL // TC  # 2 column sub-ranges of 512


def build_nc():
    nc = bacc.Bacc("TRN2", target_bir_lowering=False, debug=False, num_devices=8)
    dp = nc.declare_dram_parameter
    params = {
        "xT": dp("xT", [C, 1 + TL], f32, isOutput=False),
        # weights pre-tiled on host: cols ordered (m-group, k, col-in-tile)
        "wr_t": dp("wr_t", [P, C * C // P], bf16, isOutput=False),
        "wk_t": dp("wk_t", [P, C * C // P], bf16, isOutput=False),
        "wv_t": dp("wv_t", [P, C * C // P], bf16, isOutput=False),
        "wo_t": dp("wo_t", [P, C * C // P], bf16, isOutput=False),
        "wfk_t": dp("wfk_t", [P, C * DF // P], bf16, isOutput=False),
        "wfv_t": dp("wfv_t", [P, C * DF // P], bf16, isOutput=False),
        "wfr_t": dp("wfr_t", [P, C * C // P], bf16, isOutput=False),
        "wkcpp": dp("wkcpp", [P, H * 4], f32, isOutput=False),
        "wspp": dp("wspp", [P, NP], f32, isOutput=False),
        "smask": dp("smask", [1, 1], f32, isOutput=False),
        "tdv": dp("tdv", [1, H], f32, isOutput=False),
        "uv": dp("uv", [1, H], f32, isOutput=False),
        "yT": dp("yT", [C, TL], f32, isOutput=True),
    }
    for nm, cols in [("ln1g", NK), ("ln1b", NK), ("ln2g", NK), ("ln2b", NK),
                     ("mxk", NK), ("mxv", NK), ("mxr", NK), ("fmk", NK),
                     ("fmr", NK), ("lnxg", NP), ("lnxb", NP)]:
        params[nm] = dp(nm, [P, cols], f32, isOutput=False)
    with tile.TileContext(nc) as tc:
        _build(nc, tc, params)
    nc.compile()
    return nc


def _build(nc, tc, params):
    ctxs = []

    def pool(name, bufs, space="SBUF"):
        p = tc.tile_pool(name=name, bufs=bufs, space=space)
        ctxs.append(p)
        return p.__enter__()

    const = pool("const", 1)
    pers = pool("pers", 1)
    big = pool("big", 65)          # [P,1+TL]-bf16-slab activation tiles
    scr = pool("scr", 8)           # [P,TC]-f32 scratch
    xsrc = pool("xsrc", 2)         # [P,1+TL]-f32 streamed sources
    sscr = pool("sscr", 10)        # small [P,S] scratch
    wmtb = pool("wmtb", 5)         # [P,2TC]-bf16 cached two-head decay masks
    wts = pool("wts", 3)           # [128,2048]bf16 weight-blob ring
    psa = pool("psa", 3, space="PSUM")   # [P,2TC] f32 (2 banks)
    psb = pool("psb", 2, space="PSUM")   # [P,TC] f32 (1 bank)
    drm = pool("drm", 1, space="DRAM")

    cnt = [0]

    def bigt(dtype=bf16, cols=1 + TL):
        cnt[0] += 1
        return big.tile([P, cols], dtype, tag="big", name=f"b_{cnt[0]}")

    def sc(shape=(P, TC), dtype=f32):
        cnt[0] += 1
        return scr.tile(list(shape), dtype, tag="scr", name=f"sc_{cnt[0]}")

    def xsc():
        cnt[0] += 1
        return xsrc.tile([P, 1 + TL], f32, tag="xsrc", name=f"xs_{cnt[0]}")

    def ssc(shape=(P, S), dtype=f32):
        cnt[0] += 1
        return sscr.tile(list(shape), dtype, tag="sscr", name=f"ss_{cnt[0]}")

    def wmt_tile():
        cnt[0] += 1
        return wmtb.tile([P, 2 * TC], bf16, tag="wmtb", name=f"wm_{cnt[0]}")

    def wt_tile(cols=2048):
        cnt[0] += 1
        return wts.tile([P, cols], bf16, tag="wt", name=f"wt_{cnt[0]}")

    def psa_():
        cnt[0] += 1
        return psa.tile([P, 2 * TC], f32, tag="psa", name=f"pa_{cnt[0]}")

    def pst_(shape=(P, TC), dtype=f32):
        cnt[0] += 1
        return psb.tile(list(shape), dtype, tag="psb", name=f"pb_{cnt[0]}")

    # ---------------- constants ----------------
    IOTA_T = const.tile([P, TC], f32, tag="iota_t")
    nc.gpsimd.iota(IOTA_T[:], pattern=[[1, TC]], base=0, channel_multiplier=0,
                   allow_small_or_imprecise_dtypes=True)
    IDENT = const.tile([P, P], bf16, tag="ident")
    make_identity(nc, IDENT[:])
    IDENT2 = const.tile([P, S], bf16, tag="ident2")
    ONESPS = sc((P, S))
    nc.gpsimd.memset(ONESPS[:], 1.0)
    id2a = sc((P, S))
    nc.gpsimd.affine_select(id2a[:], ONESPS[:], pattern=[[1, S]], base=S,
                            channel_multiplier=-1, compare_op=AOT.is_ge, fill=0.0)
    nc.gpsimd.affine_select(IDENT2[:], id2a[:], pattern=[[-1, S]], base=-S,
                            channel_multiplier=1, compare_op=AOT.is_ge, fill=0.0)
    ONES_K = const.tile([P, 1], f32, tag="ones_k")
    nc.gpsimd.memset(ONES_K[:], 1.0)
    ONES_KB = const.tile([P, 1], bf16, tag="ones_kb")
    nc.gpsimd.memset(ONES_KB[:], 1.0)
    BLKPB = const.tile([P, 2], bf16, tag="blkpb")
    nc.gpsimd.memset(BLKPB[:], 0.0)
    nc.gpsimd.memset(BLKPB[0:S, 0:1], 1.0)
    nc.gpsimd.memset(BLKPB[S:P, 1:2], 1.0)
    # IOTAW[p, j*64+c] = 511 - 128*j - p  (contrib decay exponents)
    IOTAW = const.tile([P, 4 * S], f32, tag="iotaw")
    nc.gpsimd.iota(IOTAW[:], pattern=[[-P, 4], [0, S]], base=TC - 1,
                   channel_multiplier=-1, allow_small_or_imprecise_dtypes=True)
    EPSB = const.tile([P, 1], f32, tag="epsb")
    nc.gpsimd.memset(EPSB[:], EPS)
    BLKP = const.tile([P, 2], f32, tag="blkp")
    nc.gpsimd.memset(BLKP[:], 0.0)
    nc.gpsimd.memset(BLKP[0:S, 0:1], 1.0)
    nc.gpsimd.memset(BLKP[S:P, 1:2], 1.0)
    SEL2 = const.tile([2, P], f32, tag="sel2")
    ones2 = sc((2, P))
    nc.gpsimd.memset(ones2[:], 1.0)
    sel_a = sc((2, P))
    nc.gpsimd.affine_select(sel_a[:], ones2[:], pattern=[[1, P]], base=0,
                            channel_multiplier=-S, compare_op=AOT.is_ge, fill=0.0)
    nc.gpsimd.affine_select(SEL2[:], sel_a[:], pattern=[[-1, P]], base=S - 1,
                            channel_multiplier=S, compare_op=AOT.is_ge, fill=0.0)
    E4M = []
    for j in range(4):
        e = sc()
        nc.gpsimd.iota(e[:], pattern=[[1, TC]], base=-(j * P) - 1,
                       channel_multiplier=-1, allow_small_or_imprecise_dtypes=True)
        em = const.tile([P, TC], f32, tag=f"e4m_{j}", name=f"e4m_{j}")
        nc.gpsimd.affine_select(em[:], e[:], pattern=[[1, TC]], base=-(j * P) - 1,
                                channel_multiplier=-1, compare_op=AOT.is_ge, fill=1e30)
        E4M.append(em)

    def ld(name, cols):
        t = const.tile([P, cols], f32, tag=name, name=name)
        nc.sync.dma_start(t[:], params[name][:])
        return t

    LN1G = ld("ln1g", NK); LN1B = ld("ln1b", NK)
    LN2G = ld("ln2g", NK); LN2B = ld("ln2b", NK)
    MXK = ld("mxk", NK); MXV = ld("mxv", NK); MXR = ld("mxr", NK)
    FMK = ld("fmk", NK); FMR = ld("fmr", NK)
    LNXG = ld("lnxg", NP); LNXB = ld("lnxb", NP)
    WKC = ld("wkcpp", H * 4)
    WSPP = ld("wspp", NP)

    def onem(src, name):
        t = const.tile([P, NK], f32, tag=name, name=name)
        nc.vector.tensor_scalar(t[:], src[:], -1.0, 1.0, AOT.mult, AOT.add)
        return t
    MXK1 = onem(MXK, "mxk1"); MXV1 = onem(MXV, "mxv1"); MXR1 = onem(MXR, "mxr1")
    FMK1 = onem(FMK, "fmk1"); FMR1 = onem(FMR, "fmr1")

    TD = const.tile([P, H], f32, tag="td")
    nc.sync.dma_start(TD[:], params["tdv"][0:1, :].partition_broadcast(P))
    UU = const.tile([P, H], f32, tag="uu")
    nc.sync.dma_start(UU[:], params["uv"][0:1, :].partition_broadcast(P))
    SMB = const.tile([P, 1], f32, tag="smb")
    nc.sync.dma_start(SMB[:], params["smask"][0:1, :].partition_broadcast(P))
    NEGLNW = const.tile([P, H], f32, tag="neglnw")
    nc.scalar.activation(NEGLNW[:], TD[:], AFT.Exp)
    LNW = const.tile([P, H], f32, tag="lnw")
    nc.vector.tensor_scalar_mul(LNW[:], NEGLNW[:], -1.0)

    xT = params["xT"]; yT = params["yT"]

    # DRAM tiles: collectives + x' spill
    sout_d = drm.tile([P, NP * S], f32, tag="soutd")
    sgat_d = drm.tile([2 * P, NP * S], f32, tag="sgatd")
    xcol_d = drm.tile([P, NK], f32, tag="xcold")
    xcgat_d = drm.tile([2 * P, NK], f32, tag="xcgatd")
    xprime_d = drm.tile([C, TL], bf16, tag="xprd")

    # column sub-ranges of the local [*, 1+TL] tensors: halo + 2x512
    RANGES = [(0, 1)] + [(1 + i * TC, TC) for i in range(TS)]

    # ---------- layernorm over channel dim for a streamed source ----------
    def ln_pass(src_fn, g, b, dst_tiles, halo_mask):
        """src_fn(k) -> [P, 1+TL] f32 tile (fresh each call; called twice
        per k). Writes normalized bf16 into dst_tiles[k] ([P, 1+TL])."""
        stats = []  # per range: (brs, bmrs) broadcast tiles
        pssA = psa_()   # rows 0:1; cols ts*TC per main range
        psqA = psa_()
        pssh = pst_((1, 1)); psqh = pst_((1, 1))
        psr = [(pssh[:], psqh[:])] + \
              [(pssA[0:1, i * TC:(i + 1) * TC], psqA[0:1, i * TC:(i + 1) * TC])
               for i in range(TS)]
        for k in range(NK):
            t = src_fn(k)
            for ri, (off, ln) in enumerate(RANGES):
                pss, psq = psr[ri]
                sq = sc((P, ln))
                nc.vector.tensor_mul(sq[:], t[:, off:off + ln], t[:, off:off + ln])
                nc.tensor.matmul(pss, ONES_K[:], t[:, off:off + ln],
                                 start=(k == 0), stop=(k == NK - 1))
                nc.tensor.matmul(psq, ONES_K[:], sq[:],
                                 start=(k == 0), stop=(k == NK - 1))
        for ri, (off, ln) in enumerate(RANGES):
            pss, psq = psr[ri]
            m_ = sc((1, ln)); nc.scalar.mul(m_[:], pss, 1.0 / C)
            q_ = sc((1, ln)); nc.scalar.mul(q_[:], psq, 1.0 / C)
            msq = sc((1, ln)); nc.scalar.square(msq[:], m_[:])
            var = sc((1, ln)); nc.vector.tensor_sub(var[:], q_[:], msq[:])
            lnv = sc((1, ln))
            nc.scalar.activation(lnv[:], var[:], AFT.Ln, bias=EPSB[0:1, 0:1])
            rs = sc((1, ln))
            nc.scalar.activation(rs[:], lnv[:], AFT.Exp, scale=-0.5)
            mrs = sc((1, ln))
            nc.vector.scalar_tensor_tensor(mrs[:], m_[:], -1.0, rs[:],
                                           AOT.mult, AOT.mult)
            brs = sc((P, ln)); nc.gpsimd.partition_broadcast(brs[:], rs[:])
            bmrs = sc((P, ln)); nc.gpsimd.partition_broadcast(bmrs[:], mrs[:])
            stats.append((brs, bmrs))
        for k in range(NK):
            t = src_fn(k)
            dst = dst_tiles[k]
            for ri, (off, ln) in enumerate(RANGES):
                brs, bmrs = stats[ri]
                tmp = sc((P, ln))
                nc.vector.tensor_mul(tmp[:], t[:, off:off + ln], brs[:])
                nc.vector.tensor_add(tmp[:], tmp[:], bmrs[:])
                nc.vector.tensor_scalar(dst[:, off:off + ln], tmp[:],
                                        g[:, k:k + 1], b[:, k:k + 1],
                                        AOT.mult, AOT.add)
            if halo_mask:
                nc.vector.tensor_scalar(dst[:, 0:1], dst[:, 0:1],
                                        SMB[:, 0:1], None, AOT.mult)

    def mix(dst, h, cf, cf1, k):
        """dst[:, 0:TL] = cf[k]*h[:, 1:1+TL] + cf1[k]*h[:, 0:TL]"""
        nc.vector.tensor_scalar(dst[:, 0:TL], h[:, 1:1 + TL], cf[:, k:k + 1],
                                None, AOT.mult)
        nc.vector.scalar_tensor_tensor(dst[:, 0:TL], h[:, 0:TL], cf1[:, k:k + 1],
                                       dst[:, 0:TL], AOT.mult, AOT.add)

    # ---------- GEMM helper: out[m] = sum_k w_tiled[.,m,k] ^T @ in[k] ----------
    def gemm_std(wt_dram, in_tiles, out_tiles, n_out, act=None, accum=False,
                 n_in=NK, G=8, col_base=0):
        """Host-pretiled weights: blob cols ordered (mg, k, 256). m-groups
        of 2; per group ceil(n_in/G) blob DMAs of [128, G*256]; two 2-bank
        PSUM tiles (one per mi), double-buffered across groups.
        act: None -> copy; 'sigmoid'; 'sqrelu'. accum: add into out."""
        nh = (n_in + G - 1) // G
        for mgl in range(n_out // 2):
            base = col_base + mgl * n_in * 2 * P
            wtl = []
            for hf in range(nh):
                kn = min(G, n_in - hf * G)
                w = wt_tile(kn * 2 * P)
                nc.sync.dma_start(
                    w[:], wt_dram[:, base + hf * G * 2 * P:
                                  base + (hf * G + kn) * 2 * P])
                wtl.append(w)
            pq = [psa_() for _ in range(2)]
            for k in range(n_in):
                w = wtl[k // G]
                co = (k % G) * 2 * P
                for mi in range(2):
                    for ts in range(TS):
                        nc.tensor.matmul(
                            pq[mi][:, ts * TC:(ts + 1) * TC],
                            w[:, co + mi * P:co + (mi + 1) * P],
                            in_tiles[k][:, ts * TC:(ts + 1) * TC],
                            start=(k == 0), stop=(k == n_in - 1))
            for mi in range(2):
                m = mgl * 2 + mi
                dst = out_tiles[m][:, 0:TL]
                src = pq[mi][0:P, 0:TL]
                if act == "sigmoid":
                    nc.scalar.activation(dst, src, AFT.Sigmoid)
                elif act == "sqrelu":
                    rl = xsc()
                    nc.scalar.activation(rl[:, 0:TL], src, AFT.Relu)
                    nc.vector.tensor_mul(dst, rl[:, 0:TL], rl[:, 0:TL])
                elif accum:
                    nc.vector.tensor_add(dst, dst, src)
                else:
                    nc.vector.tensor_copy(dst, src)

    # ================= phase 1: LN1 =================
    def src_x(k):
        t = xsc()
        nc.sync.dma_start(t[:], xT[k * P:(k + 1) * P, :])
        return t

    H1 = [bigt() for _ in range(NK)]
    ln_pass(src_x, LN1G, LN1B, H1, halo_mask=True)

    # ================= phase 2: K / V projections =================
    MQ = [bigt(cols=TL) for _ in range(NK)]
    for k in range(NK):
        mix(MQ[k], H1[k], MXK, MXK1, k)
    KT = [bigt(cols=TL) for _ in range(NK)]
    gemm_std(params["wk_t"], MQ, KT, NK)
    for k in range(NK):
        mix(MQ[k], H1[k], MXV, MXV1, k)
    # V transposed: VT[tslab (8 x 128 tokens)] as 2 tiles of [P, 1024] each.
    # wv_t blob cols ordered (cb, k, 512); stationary = MQ slab, moving = w.
    VT = [[bigt(cols=TL) for _ in range(2)] for _ in range(2 * 4)]
    for tg in range(2):       # t-slab groups of 4
        for cb in range(4):   # c_out banks of 512
            pv = [psa_() for _ in range(2)]
            for kq in range(4):
                w = wt_tile()
                nc.sync.dma_start(
                    w[:], params["wv_t"][:, (cb * NK + kq * 4) * TC:
                                         (cb * NK + kq * 4 + 4) * TC])
                for kk in range(4):
                    k = kq * 4 + kk
                    for ti in range(4):
                        tslab = tg * 4 + ti
                        nc.tensor.matmul(
                            pv[ti // 2][:, (ti % 2) * TC:(ti % 2 + 1) * TC],
                            MQ[k][:, tslab * P:(tslab + 1) * P],
                            w[:, kk * TC:(kk + 1) * TC],
                            start=(k == 0), stop=(k == NK - 1))
            for ti in range(4):
                nc.vector.tensor_copy(
                    VT[tg * 4 + ti][cb // 2][:, (cb % 2) * TC:(cb % 2 + 1) * TC],
                    pv[ti // 2][:, (ti % 2) * TC:(ti % 2 + 1) * TC])

    def vsl(i, j, h):
        """[P,S] value slice: chunk i, 128-token slab j, head h."""
        voff = h * S
        return VT[i * 4 + j][voff // TL][:, voff % TL:voff % TL + S]

    # ============ phase 3: state contributions + AllGather ============
    CONTRIB0 = [pers.tile([P, S], f32, tag=f"c0_{p}", name=f"c0_{p}")
                for p in range(NP)]
    for p in range(NP):
        wkct = []
        for hh in range(2):
            h = 2 * p + hh
            t = sc((P, 4 * S), bf16)
            nc.scalar.activation(t[:], IOTAW[:], AFT.Exp,
                                 scale=LNW[:, h:h + 1])
            wkct.append(t)
        cts = []
        for i in range(NCH):
            pst = pst_((P, S))
            for hh in range(2):
                h = 2 * p + hh
                pr = slice(hh * S, hh * S + S)
                ident = IDENT[0:S, 0:S] if hh == 0 else IDENT2[S:P, :]
                ptr4 = pst_((P, 4 * S), bf16)
                for j in range(4):
                    nc.tensor.transpose(
                        ptr4[:, j * S:(j + 1) * S],
                        KT[p][pr, i * TC + j * P:i * TC + (j + 1) * P],
                        ident)
                kkwt = sc((P, 4 * S), bf16)
                nc.vector.tensor_mul(kkwt[:], ptr4[:], wkct[hh][:])
                for j in range(4):
                    nc.tensor.matmul(pst[pr, :], kkwt[:, j * S:(j + 1) * S],
                                     vsl(i, j, h),
                                     start=(j == 0), stop=(j == 3))
            if i == 0:
                nc.vector.tensor_copy(CONTRIB0[p][:], pst[:])
                cts.append(CONTRIB0[p])
            else:
                c1 = ssc()
                nc.vector.tensor_copy(c1[:], pst[:])
                cts.append(c1)
        so = ssc()
        nc.vector.scalar_tensor_tensor(so[:], cts[0][:], WSPP[:, p:p + 1],
                                       cts[1][:], AOT.mult, AOT.add)
        nc.sync.dma_start(sout_d[:, p * S:(p + 1) * S], so[:])
    nc.gpsimd.collective_compute("AllGather", AOT.bypass, replica_groups=GROUPS,
                                 ins=[sout_d.opt()], outs=[sgat_d.opt()])

    # ================= phase 4: R projection (overlaps AG) =================
    for k in range(NK):
        mix(MQ[k], H1[k], MXR, MXR1, k)
    H1 = None
    RT = [bigt(cols=TL) for _ in range(NK)]
    gemm_std(params["wr_t"], MQ, RT, NK)
    MQ = None

    # incoming state = smask * (rank0 shard of gather)
    SIN = pers.tile([P, NP * S], f32, tag="sin", name="sin")
    nc.sync.dma_start(SIN[:], sgat_d[0:P, :])
    nc.vector.tensor_scalar(SIN[:], SIN[:], SMB[:, 0:1], None, AOT.mult)

    # ================= phase 5: attention + groupnorm =================
    # Pair-outer loop: decay masks (wmt) and wb computed once per pair and
    # reused for both chunks. Groupnorm stats gathered into [64, TC] tiles
    # (rows 2p:2p+2 = sums, rows 32+2p = sum-of-squares) and normalized in
    # one batched pass per chunk.
    XA = [bigt(cols=TL) for _ in range(NK)]
    for p in range(NP):
        wb = sc()
        nc.scalar.activation(wb[0:S, :], IOTA_T[0:S, :], AFT.Exp,
                             scale=LNW[0:S, 2 * p:2 * p + 1])
        nc.scalar.activation(wb[S:P, :], IOTA_T[S:P, :], AFT.Exp,
                             scale=LNW[S:P, 2 * p + 1:2 * p + 2])
        WMT = []
        for j in range(4):
            wmt = wmt_tile()   # cols 0:TC = head 2p, TC:2TC = head 2p+1
            for hh in range(2):
                h = 2 * p + hh
                nc.scalar.activation(wmt[:, hh * TC:(hh + 1) * TC], E4M[j][:],
                                     AFT.Exp, scale=LNW[:, h:h + 1])
                nc.vector.scalar_tensor_tensor(
                    wmt[:, hh * TC + j * P:hh * TC + (j + 1) * P],
                    IDENT[:, 0:P], UU[:, h:h + 1],
                    wmt[:, hh * TC + j * P:hh * TC + (j + 1) * P],
                    AOT.mult, AOT.add)
            WMT.append(wmt)
        for i in range(NCH):
            st_mm = ssc((P, S), bf16)
            if i == 0:
                nc.vector.tensor_copy(st_mm[:], SIN[:, p * S:(p + 1) * S])
            else:
                stt = ssc()
                nc.vector.scalar_tensor_tensor(stt[:], SIN[:, p * S:(p + 1) * S],
                                               WSPP[:, p:p + 1], CONTRIB0[p][:],
                                               AOT.mult, AOT.add)
                nc.vector.tensor_copy(st_mm[:], stt[:])
            rtw = sc(dtype=bf16)
            nc.vector.tensor_mul(rtw[:], RT[p][:, i * TC:(i + 1) * TC], wb[:])
            pout = pst_((P, TC))
            for hh in range(2):
                pr = slice(hh * S, hh * S + S)
                nc.tensor.matmul(pout[pr, :], st_mm[pr, :], rtw[pr, :],
                                 start=True, stop=False)
            for j in range(4):
                pa2 = psa_()
                for hh in range(2):
                    pr = slice(hh * S, hh * S + S)
                    nc.tensor.matmul(
                        pa2[:, hh * TC:(hh + 1) * TC],
                        KT[p][pr, i * TC + j * P:i * TC + (j + 1) * P],
                        RT[p][pr, i * TC:(i + 1) * TC],
                        start=True, stop=True)
                ast = sc((P, 2 * TC), bf16)
                nc.vector.tensor_mul(ast[:], pa2[:], WMT[j][:])
                for hh in range(2):
                    h = 2 * p + hh
                    pr = slice(hh * S, hh * S + S)
                    nc.tensor.matmul(pout[pr, :], vsl(i, j, h),
                                     ast[:, hh * TC:(hh + 1) * TC],
                                     start=False, stop=(j == 3))
            # groupnorm (per pair, Rsqrt path); XA gets normalized bf16
            nc.vector.tensor_copy(XA[p][:, i * TC:(i + 1) * TC], pout[:])
            sq = sc()
            nc.vector.tensor_mul(sq[:], XA[p][:, i * TC:(i + 1) * TC],
                                 XA[p][:, i * TC:(i + 1) * TC])
            pgs = pst_((2, TC))
            nc.tensor.matmul(pgs[:], BLKPB[:], XA[p][:, i * TC:(i + 1) * TC],
                             start=True, stop=True)
            pgq = pst_((2, TC))
            nc.tensor.matmul(pgq[:], BLKP[:], sq[:], start=True, stop=True)
            m_ = sc((2, TC)); nc.scalar.mul(m_[:], pgs[:], 1.0 / (S * HS_DIV))
            q_ = sc((2, TC)); nc.scalar.mul(q_[:], pgq[:], 1.0 / (S * HS_DIV * HS_DIV))
            msq = sc((2, TC)); nc.vector.tensor_mul(msq[:], m_[:], m_[:])
            var = sc((2, TC)); nc.vector.tensor_sub(var[:], q_[:], msq[:])
            lnv = sc((2, TC))
            nc.scalar.activation(lnv[:], var[:], AFT.Ln, bias=EPSB[0:2, 0:1])
            rs = sc((2, TC))
            nc.scalar.activation(rs[:], lnv[:], AFT.Exp, scale=-0.5)
            mrs = sc((2, TC))
            nc.vector.scalar_tensor_tensor(mrs[:], m_[:], -1.0, rs[:],
                                           AOT.mult, AOT.mult)
            rsh = sc((2, TC)); nc.vector.tensor_scalar_mul(rsh[:], rs[:], 1.0 / HS_DIV)
            pbr = pst_((P, TC))
            nc.tensor.matmul(pbr[:], SEL2[:], rsh[:], start=True, stop=True)
            pbm = pst_((P, TC))
            nc.tensor.matmul(pbm[:], SEL2[:], mrs[:], start=True, stop=True)
            xa = sc()
            nc.vector.tensor_mul(xa[:], XA[p][:, i * TC:(i + 1) * TC], pbr[:])
            nc.vector.tensor_add(xa[:], xa[:], pbm[:])
            nc.vector.tensor_scalar(XA[p][:, i * TC:(i + 1) * TC], xa[:],
                                    LNXG[:, p:p + 1], LNXB[:, p:p + 1],
                                    AOT.mult, AOT.add)
    RT = KT = VT = None

    # ================= phase 6: Wo + residual, spill x' =================
    XP = [bigt(cols=TL) for _ in range(NK)]
    gemm_std(params["wo_t"], XA, XP, NK)
    XA = None
    for k in range(NK):
        xr = xsc()
        nc.sync.dma_start(xr[:, :], xT[k * P:(k + 1) * P, :])
        for ts in range(TS):
            xpf = sc()
            nc.vector.tensor_add(xpf[:], XP[k][:, ts * TC:(ts + 1) * TC],
                                 xr[:, 1 + ts * TC:1 + (ts + 1) * TC])
            nc.vector.tensor_copy(XP[k][:, ts * TC:(ts + 1) * TC], xpf[:])
            nc.sync.dma_start(xprime_d[k * P:(k + 1) * P, ts * TC:(ts + 1) * TC],
                              XP[k][:, ts * TC:(ts + 1) * TC])
            if ts == TS - 1:
                lc = sc((P, 1))
                nc.vector.tensor_copy(lc[:], xpf[:, TC - 1:TC])
                nc.sync.dma_start(xcol_d[:, k:k + 1], lc[:])
    nc.gpsimd.collective_compute("AllGather", AOT.bypass, replica_groups=GROUPS,
                                 ins=[xcol_d.opt()], outs=[xcgat_d.opt()])

    # ================= phase 7: LN2 + mixes =================
    XCH = const.tile([P, NK], f32, tag="xch")   # per-chunk halo cols
    nc.sync.dma_start(XCH[:], xcgat_d[0:P, :])

    H2 = [bigt() for _ in range(NK)]

    def src_x2(k):
        t = xsc()
        nc.vector.tensor_copy(t[:, 0:1], XCH[:, k:k + 1])
        nc.vector.tensor_copy(t[:, 1:1 + TL], XP[k][:, 0:TL])
        return t

    ln_pass(src_x2, LN2G, LN2B, H2, halo_mask=True)
    XP = None
    MFK = [bigt(cols=TL) for _ in range(NK)]
    MFR = [bigt(cols=TL) for _ in range(NK)]
    for k in range(NK):
        mix(MFK[k], H2[k], FMK, FMK1, k)
        mix(MFR[k], H2[k], FMR, FMR1, k)
    H2 = None

    # ================= phase 8: gate = sigmoid(mfr @ wfr) =================
    GT = [bigt(cols=TL) for _ in range(NK)]
    gemm_std(params["wfr_t"], MFR, GT, NK, act="sigmoid")
    MFR = None

    # ========== phase 9: FFN quarters: kf=relu^2(mfk@wfk); kv+=wfv^T@kf ==========
    KV = [bigt(cols=TL) for _ in range(NK)]
    for q in range(NQ):
        KF = [bigt(cols=TL) for _ in range(JQ)]
        gemm_std(params["wfk_t"], MFK, KF, JQ, act="sqrelu",
                 col_base=q * (JQ // 2) * NK * 2 * P)
        # kv partial: contract the quarter's 14 j-chunks
        gemm_std(params["wfv_t"], KF, KV, NK, accum=(q > 0), n_in=JQ, G=7,
                 col_base=q * (NK // 2) * JQ * 2 * P)
        KF = None
    MFK = None

    # ================= phase 10: y = x' + gate*kv =================
    for k in range(NK):
        for ts in range(TS):
            xp = sc(dtype=bf16)
            nc.sync.dma_start(xp[:], xprime_d[k * P:(k + 1) * P,
                                              ts * TC:(ts + 1) * TC])
            gk = sc()
            nc.vector.tensor_mul(gk[:], GT[k][:, ts * TC:(ts + 1) * TC],
                                 KV[k][:, ts * TC:(ts + 1) * TC])
            yo = sc()
            nc.vector.tensor_add(yo[:], xp[:], gk[:])
            nc.sync.dma_start(yT[k * P:(k + 1) * P, ts * TC:(ts + 1) * TC], yo[:])

    for c in reversed(ctxs):
        c.__exit__(None, None, None)


# ----------------------------------------------------------------------
# Host-side sharding / gather
# ----------------------------------------------------------------------
import ml_dtypes

_NC_CACHE = {}


def _vec_pk(v, nk=NK):
    return np.ascontiguousarray(np.asarray(v).reshape(nk, P).T.astype(np.float32))


def _make_in_maps(inputs):
    x = np.asarray(inputs["x"], np.float32)
    bf = ml_dtypes.bfloat16
    td = np.asarray(inputs["time_decay"], np.float64)
    w = np.exp(-np.exp(td))                      # [H]
    ws = w ** TC
    wspp = np.zeros((P, NP), np.float32)
    for p in range(NP):
        wspp[0:S, p] = ws[2 * p]
        wspp[S:P, p] = ws[2 * p + 1]
    wkcpp = np.zeros((P, H * 4), np.float32)
    pp = np.arange(P)
    for h in range(H):
        for j in range(4):
            wkcpp[:, h * 4 + j] = w[h] ** (TC - 1 - j * P - pp)
    def _tile_mk(W):
        """[n_in*128, n_mg*256] -> [128, n_mg*n_in*256], cols (mg, k, c)."""
        n_in = W.shape[0] // P
        n_mg = W.shape[1] // (2 * P)
        return np.ascontiguousarray(
            W.reshape(n_in, P, n_mg, 2 * P).transpose(1, 2, 0, 3)
             .reshape(P, -1).astype(bf))

    wcache = {}
    for nm, key in [("wr_t", "Wr"), ("wk_t", "Wk"), ("wo_t", "Wo"),
                    ("wfr_t", "Wfr"), ("wfk_t", "Wfk")]:
        wcache[nm] = _tile_mk(np.asarray(inputs[key], np.float32))
    Wfv = np.asarray(inputs["Wfv"], np.float32)
    wcache["wfv_t"] = np.ascontiguousarray(np.concatenate(
        [_tile_mk(Wfv[q * JQ * P:(q + 1) * JQ * P, :]) for q in range(NQ)],
        axis=1))
    Wv = np.asarray(inputs["Wv"], np.float32)
    wcache["wv_t"] = np.ascontiguousarray(
        Wv.reshape(NK, P, 4, TC).transpose(1, 2, 0, 3).reshape(P, -1).astype(bf))
    maps = []
    for c in range(8):
        b, half = c // 2, c % 2
        t0 = half * TL
        xh = np.zeros((C, 1 + TL), np.float32)
        xh[:, 1:] = x[b, t0:t0 + TL, :].T
        if half == 1:
            xh[:, 0] = x[b, t0 - 1, :]
        maps.append({
            "xT": np.ascontiguousarray(xh),
            **wcache,
            "wkcpp": wkcpp, "wspp": wspp,
            "smask": np.full((1, 1), float(half), np.float32),
            "ln1g": _vec_pk(inputs["ln1_g"]), "ln1b": _vec_pk(inputs["ln1_b"]),
            "ln2g": _vec_pk(inputs["ln2_g"]), "ln2b": _vec_pk(inputs["ln2_b"]),
            "mxk": _vec_pk(inputs["att_mix_k"]), "mxv": _vec_pk(inputs["att_mix_v"]),
            "mxr": _vec_pk(inputs["att_mix_r"]),
            "fmk": _vec_pk(inputs["ffn_mix_k"]), "fmr": _vec_pk(inputs["ffn_mix_r"]),
            "lnxg": _vec_pk(inputs["lnx_g"], NP),
            "lnxb": _vec_pk(inputs["lnx_b"], NP),
            "tdv": np.ascontiguousarray(np.asarray(inputs["time_decay"],
                                                   np.float32)[None, :]),
            "uv": np.ascontiguousarray(np.asarray(inputs["time_faaaa"],
                                                  np.float32)[None, :]),
        })
    return maps


def run_on_hw(inputs, trace=False):
    from concourse.bass_utils import run_bass_kernel_spmd
    if "nc" not in _NC_CACHE:
        _NC_CACHE["nc"] = build_nc()
    nc = _NC_CACHE["nc"]
    maps = _make_in_maps(inputs)
    res = run_bass_kernel_spmd(nc, maps, core_ids=list(range(8)), trace=trace)
    B = 4
    out = np.zeros((B, 2 * TL, C), np.float32)
    for c in range(8):
        b, half = c // 2, c % 2
        out[b, half * TL:(half + 1) * TL, :] = res.results[c]["yT"].T
    return out, res


def kernel(**inputs) -> np.ndarray:
    out, _ = run_on_hw(inputs, trace=False)
    return out



# revision 9
# speedup vs baseline: 1.1807x; 1.1807x over previous
"""RWKV5 block, sequence-parallel across 8 trn2 cores.

Core c -> batch c//2, sequence half c%2 (tokens t0 = half*1024, TL=1024
= 2 recurrence chunks of TC=512). Each core runs FULL-width GEMMs
(C=2048, DF=7168) on its token half; every weight is streamed from HBM
once (Wv twice). Cross-core traffic per pair: one 512KB state AllGather
(recurrent state after chunk 1 -> second half) plus an 8KB x' halo
column AllGather for the ChannelMix time-shift.

Layout: activations channel-major [C, T]. v kept time-major [T, C]
(VT) for the attention a@v and k^T@v contractions.

v2: scheduling-focused rewrite. Attention decay masks are built as
column-shifted views of one per-head exp table (M) plus a diagonal
block tile (D), pa2/pout matmuls are column-restricted to the nonzero
mask region, groupnorm is deferred into a batched per-chunk pass with
gpsimd partition-broadcasts (no fp32 matmuls, no per-pair table
swaps), LN stats run on bf16 operands, and the final FFN quarter
writes y directly.
"""
import numpy as np
import concourse.bass as bass
import concourse.mybir as mybir
import concourse.tile as tile
from concourse import bacc
from concourse.masks import make_identity

f32 = mybir.dt.float32
bf16 = mybir.dt.bfloat16
AOT = mybir.AluOpType
AFT = mybir.ActivationFunctionType

C = 2048
H = 32         # heads
S = 64         # head dim
TC = 512       # recurrence chunk
TL = 1024      # local tokens per core
NCH = TL // TC # 2 local chunks
DF = 7168
P = 128
NK = C // P    # 16 channel chunks
NP = H // 2    # 16 head pairs
NJ = DF // P   # 56
NQ = 4         # DF quarters
JQ = NJ // NQ  # 14 j-chunks per quarter
EPS = 1e-5
HS_DIV = float(np.sqrt(S))
GROUPS = [[0, 1], [2, 3], [4, 5], [6, 7]]
TS = TL // TC  # 2 column sub-ranges of 512


def build_nc():
    nc = bacc.Bacc("TRN2", target_bir_lowering=False, debug=False, num_devices=8)
    dp = nc.declare_dram_parameter
    params = {
        "xT": dp("xT", [C, 1 + TL], f32, isOutput=False),
        # weights pre-tiled on host: cols ordered (m-group, k, col-in-tile)
        "wr_t": dp("wr_t", [P, C * C // P], bf16, isOutput=False),
        "wk_t": dp("wk_t", [P, C * C // P], bf16, isOutput=False),
        "wv_t": dp("wv_t", [P, C * C // P], bf16, isOutput=False),
        "wo_t": dp("wo_t", [P, C * C // P], bf16, isOutput=False),
        "wfk_t": dp("wfk_t", [P, C * DF // P], bf16, isOutput=False),
        "wfv_t": dp("wfv_t", [P, C * DF // P], bf16, isOutput=False),
        "wfr_t": dp("wfr_t", [P, C * C // P], bf16, isOutput=False),
        "wspp": dp("wspp", [P, NP], f32, isOutput=False),
        "smask": dp("smask", [1, 1], f32, isOutput=False),
        "tdv": dp("tdv", [1, H], f32, isOutput=False),
        "uv": dp("uv", [1, H], f32, isOutput=False),
        "yT": dp("yT", [C, TL], f32, isOutput=True),
    }
    for nm, cols in [("ln1g", NK), ("ln1b", NK), ("ln2g", NK), ("ln2b", NK),
                     ("mxk", NK), ("mxv", NK), ("mxr", NK), ("fmk", NK),
                     ("fmr", NK), ("lnxg", NP), ("lnxb", NP)]:
        params[nm] = dp(nm, [P, cols], f32, isOutput=False)
    with tile.TileContext(nc) as tc:
        _build(nc, tc, params)
    nc.compile()
    return nc


def _build(nc, tc, params):
    ctxs = []

    def pool(name, bufs, space="SBUF"):
        p = tc.tile_pool(name=name, bufs=bufs, space=space)
        ctxs.append(p)
        return p.__enter__()

    const = pool("const", 1)
    pers = pool("pers", 1)
    big = pool("big", 65)          # [P,1+TL]-bf16-slab activation tiles
    scr = pool("scr", 9)           # [P,2TC]-slab scratch
    xsrc = pool("xsrc", 3)         # [P,1+TL]-f32 streamed sources
    sscr = pool("sscr", 10)        # small [P,S] scratch
    mtb = pool("mtb", 4)           # [P,TC]-bf16 per-head decay tables M
    dtb = pool("dtb", 4)           # [P,P]-bf16 diagonal-block masks D
    wts = pool("wts", 3)           # [128,2048]bf16 weight-blob ring
    psa = pool("psa", 3, space="PSUM")   # [P,2TC] f32 (2 banks)
    psb = pool("psb", 2, space="PSUM")   # [P,TC] f32 (1 bank)
    drm = pool("drm", 1, space="DRAM")

    cnt = [0]

    def bigt(dtype=bf16, cols=1 + TL):
        cnt[0] += 1
        return big.tile([P, cols], dtype, tag="big", name=f"b_{cnt[0]}")

    def sc(shape=(P, TC), dtype=f32):
        cnt[0] += 1
        return scr.tile(list(shape), dtype, tag="scr", name=f"sc_{cnt[0]}")

    def xsc():
        cnt[0] += 1
        return xsrc.tile([P, 1 + TL], f32, tag="xsrc", name=f"xs_{cnt[0]}")

    def ssc(shape=(P, S), dtype=f32):
        cnt[0] += 1
        return sscr.tile(list(shape), dtype, tag="sscr", name=f"ss_{cnt[0]}")

    def mt_tile():
        cnt[0] += 1
        return mtb.tile([P, TC], bf16, tag="mtb", name=f"mt_{cnt[0]}")

    def dt_tile():
        cnt[0] += 1
        return dtb.tile([P, P], bf16, tag="dtb", name=f"dt_{cnt[0]}")

    def wt_tile(cols=2048):
        cnt[0] += 1
        return wts.tile([P, cols], bf16, tag="wt", name=f"wt_{cnt[0]}")

    def psa_():
        cnt[0] += 1
        return psa.tile([P, 2 * TC], f32, tag="psa", name=f"pa_{cnt[0]}")

    def pst_(shape=(P, TC), dtype=f32):
        cnt[0] += 1
        return psb.tile(list(shape), dtype, tag="psb", name=f"pb_{cnt[0]}")

    # ---------------- constants ----------------
    IOTA_T = const.tile([P, TC], f32, tag="iota_t")
    nc.gpsimd.iota(IOTA_T[:], pattern=[[1, TC]], base=0, channel_multiplier=0,
                   allow_small_or_imprecise_dtypes=True)
    IDENT = const.tile([P, P], bf16, tag="ident")
    make_identity(nc, IDENT[:])
    ONES_KB = const.tile([P, 1], bf16, tag="ones_kb")
    nc.gpsimd.memset(ONES_KB[:], 1.0)
    # GSEL[ch, c]: rows 0:64 set at col 30, rows 64:128 at col 31. Slicing
    # GSEL[:, 30-2p : 62-2p] yields a [128, 32] selector whose matmul
    # accumulates pair p's per-head column sums into rows 2p:2p+2.
    GSEL = const.tile([P, S - 2], bf16, tag="gsel")
    nc.gpsimd.memset(GSEL[:], 0.0)
    nc.gpsimd.memset(GSEL[0:S, 30:31], 1.0)
    nc.gpsimd.memset(GSEL[S:P, 31:32], 1.0)
    # IOTAW[p, j*64+c] = 511 - 128*j - p  (contrib decay exponents)
    IOTAW = const.tile([P, 4 * S], f32, tag="iotaw")
    nc.gpsimd.iota(IOTAW[:], pattern=[[-P, 4], [0, S]], base=TC - 1,
                   channel_multiplier=-1, allow_small_or_imprecise_dtypes=True)
    EPSB = const.tile([P, 1], f32, tag="epsb")
    nc.gpsimd.memset(EPSB[:], EPS)
    # IOTA_MM[p, c] = c - p where c >= p else +1e30 (off-diag decay table)
    IOTA_MM = const.tile([P, TC], f32, tag="iota_mm")
    imm_raw = sc()
    nc.gpsimd.iota(imm_raw[:], pattern=[[1, TC]], base=0, channel_multiplier=-1,
                   allow_small_or_imprecise_dtypes=True)
    nc.gpsimd.affine_select(IOTA_MM[:], imm_raw[:], pattern=[[1, TC]], base=0,
                            channel_multiplier=-1, compare_op=AOT.is_ge,
                            fill=1e30)
    # IOTA_DD[p, q] = q - p - 1 where q > p else +1e30 (diag-block decay)
    IOTA_DD = const.tile([P, P], f32, tag="iota_dd")
    idd_raw = sc((P, P))
    nc.gpsimd.iota(idd_raw[:], pattern=[[1, P]], base=-1, channel_multiplier=-1,
                   allow_small_or_imprecise_dtypes=True)
    nc.gpsimd.affine_select(IOTA_DD[:], idd_raw[:], pattern=[[1, P]], base=-1,
                            channel_multiplier=-1, compare_op=AOT.is_ge,
                            fill=1e30)

    def ld(name, cols):
        t = const.tile([P, cols], f32, tag=name, name=name)
        nc.sync.dma_start(t[:], params[name][:])
        return t

    LN1G = ld("ln1g", NK); LN1B = ld("ln1b", NK)
    LN2G = ld("ln2g", NK); LN2B = ld("ln2b", NK)
    MXK = ld("mxk", NK); MXV = ld("mxv", NK); MXR = ld("mxr", NK)
    FMK = ld("fmk", NK); FMR = ld("fmr", NK)
    LNXG = ld("lnxg", NP); LNXB = ld("lnxb", NP)
    WSPP = ld("wspp", NP)

    def onem(src, name):
        t = const.tile([P, NK], f32, tag=name, name=name)
        nc.vector.tensor_scalar(t[:], src[:], -1.0, 1.0, AOT.mult, AOT.add)
        return t
    MXK1 = onem(MXK, "mxk1"); MXV1 = onem(MXV, "mxv1"); MXR1 = onem(MXR, "mxr1")
    FMK1 = onem(FMK, "fmk1"); FMR1 = onem(FMR, "fmr1")

    TD = const.tile([P, H], f32, tag="td")
    nc.sync.dma_start(TD[:], params["tdv"][0:1, :].partition_broadcast(P))
    UU = const.tile([P, H], f32, tag="uu")
    nc.sync.dma_start(UU[:], params["uv"][0:1, :].partition_broadcast(P))
    SMB = const.tile([P, 1], f32, tag="smb")
    nc.sync.dma_start(SMB[:], params["smask"][0:1, :].partition_broadcast(P))
    NEGLNW = const.tile([P, H], f32, tag="neglnw")
    nc.scalar.activation(NEGLNW[:], TD[:], AFT.Exp)
    LNW = const.tile([P, H], f32, tag="lnw")
    nc.vector.tensor_scalar_mul(LNW[:], NEGLNW[:], -1.0)

    xT = params["xT"]; yT = params["yT"]

    # DRAM tiles: collectives + x' spill + groupnorm broadcast bounce
    sout_d = drm.tile([P, NP * S], f32, tag="soutd")
    sgat_d = drm.tile([2 * P, NP * S], f32, tag="sgatd")
    xcol_d = drm.tile([P, NK], f32, tag="xcold")
    xcgat_d = drm.tile([2 * P, NK], f32, tag="xcgatd")
    xprime_d = drm.tile([C, TL], bf16, tag="xprd")
    rs_d = [drm.tile([H, TC], bf16, tag=f"rsd_{i}", name=f"rsd_{i}")
            for i in range(NCH)]
    mr_d = [drm.tile([H, TC], bf16, tag=f"mrd_{i}", name=f"mrd_{i}")
            for i in range(NCH)]

    # ---------- layernorm over channel dim (bf16 stats + batched tables) ----
    def ln_pass(srcs, halo_src, g, b, dst_tiles, halo_mask):
        """srcs(k, ts) -> [P, TC] bf16 AP; halo_src(k) -> [P, 1] bf16 AP.
        Writes normalized bf16 into dst_tiles[k] ([P, 1+TL]; halo at col 0).
        Main ranges first; halo block issued last (hides the halo AG for
        LN2). Ln/Exp batched so the act table loads happen once each."""
        pssA = psa_()   # rows 0:1; cols ts*TC per main range
        psqA = psa_()
        for k in range(NK):
            for ts in range(TS):
                s = srcs(k, ts)
                sq = sc((P, TC), bf16)
                nc.vector.tensor_mul(sq[:], s, s)
                nc.tensor.matmul(pssA[0:1, ts * TC:(ts + 1) * TC], ONES_KB[:],
                                 s, start=(k == 0), stop=(k == NK - 1))
                nc.tensor.matmul(psqA[0:1, ts * TC:(ts + 1) * TC], ONES_KB[:],
                                 sq[:], start=(k == 0), stop=(k == NK - 1))
        vars_ = []
        for ts in range(TS):
            pss = pssA[0:1, ts * TC:(ts + 1) * TC]
            psq = psqA[0:1, ts * TC:(ts + 1) * TC]
            m_ = sc((1, TC)); nc.scalar.mul(m_[:], pss, 1.0 / C)
            q_ = sc((1, TC)); nc.scalar.mul(q_[:], psq, 1.0 / C)
            msq = sc((1, TC)); nc.vector.tensor_mul(msq[:], m_[:], m_[:])
            var = sc((1, TC)); nc.vector.tensor_sub(var[:], q_[:], msq[:])
            vars_.append((m_, var))
        lnvs = []
        for ts in range(TS):
            m_, var = vars_[ts]
            lnv = sc((1, TC))
            nc.scalar.activation(lnv[:], var[:], AFT.Ln, bias=EPSB[0:1, 0:1])
            lnvs.append(lnv)
        stats = []
        for ts in range(TS):
            m_, _ = vars_[ts]
            rs = sc((1, TC))
            nc.scalar.activation(rs[:], lnvs[ts][:], AFT.Exp, scale=-0.5)
            mrs = sc((1, TC))
            nc.vector.scalar_tensor_tensor(mrs[:], m_[:], -1.0, rs[:],
                                           AOT.mult, AOT.mult)
            brs = sc((P, TC)); nc.gpsimd.partition_broadcast(brs[:], rs[:])
            bmrs = sc((P, TC)); nc.gpsimd.partition_broadcast(bmrs[:], mrs[:])
            stats.append((brs, bmrs))
        for k in range(NK):
            dst = dst_tiles[k]
            for ts in range(TS):
                brs, bmrs = stats[ts]
                tmp = sc((P, TC))
                nc.vector.tensor_mul(tmp[:], srcs(k, ts), brs[:])
                nc.vector.tensor_add(tmp[:], tmp[:], bmrs[:])
                nc.vector.tensor_scalar(dst[:, 1 + ts * TC:1 + (ts + 1) * TC],
                                        tmp[:], g[:, k:k + 1], b[:, k:k + 1],
                                        AOT.mult, AOT.add)
        # ---- halo column (issued last; waits on AG for LN2) ----
        pssh = pst_((1, 1)); psqh = pst_((1, 1))
        for k in range(NK):
            hs = halo_src(k)
            hsq = sc((P, 1), bf16)
            nc.vector.tensor_mul(hsq[:], hs, hs)
            nc.tensor.matmul(pssh[:], ONES_KB[:], hs,
                             start=(k == 0), stop=(k == NK - 1))
            nc.tensor.matmul(psqh[:], ONES_KB[:], hsq[:],
                             start=(k == 0), stop=(k == NK - 1))
        hm = sc((1, 1)); nc.scalar.mul(hm[:], pssh[:], 1.0 / C)
        hq = sc((1, 1)); nc.scalar.mul(hq[:], psqh[:], 1.0 / C)
        hmsq = sc((1, 1)); nc.vector.tensor_mul(hmsq[:], hm[:], hm[:])
        hvar = sc((1, 1)); nc.vector.tensor_sub(hvar[:], hq[:], hmsq[:])
        hlnv = sc((1, 1))
        nc.scalar.activation(hlnv[:], hvar[:], AFT.Ln, bias=EPSB[0:1, 0:1])
        hrs = sc((1, 1))
        nc.scalar.activation(hrs[:], hlnv[:], AFT.Exp, scale=-0.5)
        hmrs = sc((1, 1))
        nc.vector.scalar_tensor_tensor(hmrs[:], hm[:], -1.0, hrs[:],
                                       AOT.mult, AOT.mult)
        hbrs = sc((P, 1)); nc.gpsimd.partition_broadcast(hbrs[:], hrs[:])
        hbmrs = sc((P, 1)); nc.gpsimd.partition_broadcast(hbmrs[:], hmrs[:])
        for k in range(NK):
            dst = dst_tiles[k]
            tmp = sc((P, 1))
            nc.vector.tensor_mul(tmp[:], halo_src(k), hbrs[:])
            nc.vector.tensor_add(tmp[:], tmp[:], hbmrs[:])
            nc.vector.tensor_scalar(dst[:, 0:1], tmp[:], g[:, k:k + 1],
                                    b[:, k:k + 1], AOT.mult, AOT.add)
            if halo_mask:
                nc.vector.tensor_scalar(dst[:, 0:1], dst[:, 0:1],
                                        SMB[:, 0:1], None, AOT.mult)

    def mix(dst, h, cf, cf1, k):
        """dst[:, 0:TL] = cf[k]*h[:, 1:1+TL] + cf1[k]*h[:, 0:TL], split
        per 512-col sub-range so ts=1 doesn't depend on the halo col."""
        for ts in range(TS):
            c0, c1 = ts * TC, (ts + 1) * TC
            nc.vector.tensor_scalar(dst[:, c0:c1], h[:, 1 + c0:1 + c1],
                                    cf[:, k:k + 1], None, AOT.mult)
            nc.vector.scalar_tensor_tensor(dst[:, c0:c1], h[:, c0:c1],
                                           cf1[:, k:k + 1], dst[:, c0:c1],
                                           AOT.mult, AOT.add)

    # ---------- GEMM helper: out[m] = sum_k w_tiled[.,m,k] ^T @ in[k] ----------
    def gemm_std(wt_dram, in_tiles, out_tiles, n_out, act=None, accum=False,
                 n_in=NK, G=8, col_base=0, post=None, pre=None):
        """Host-pretiled weights: blob cols ordered (mg, k, 256). m-groups
        of 2; per group ceil(n_in/G) blob DMAs of [128, G*256]; two 2-bank
        PSUM tiles (one per mi), double-buffered across groups.
        act: None -> copy; 'sigmoid'; 'sqrelu'. accum: add into out.
        post(m, pq): custom finalize; pre(mgl): prefetch hook."""
        nh = (n_in + G - 1) // G
        for mgl in range(n_out // 2):
            if pre is not None:
                pre(mgl)
            base = col_base + mgl * n_in * 2 * P
            wtl = []
            for hf in range(nh):
                kn = min(G, n_in - hf * G)
                w = wt_tile(kn * 2 * P)
                nc.sync.dma_start(
                    w[:], wt_dram[:, base + hf * G * 2 * P:
                                  base + (hf * G + kn) * 2 * P])
                wtl.append(w)
            pq = [psa_() for _ in range(2)]
            for k in range(n_in):
                w = wtl[k // G]
                co = (k % G) * 2 * P
                for mi in range(2):
                    for ts in range(TS):
                        nc.tensor.matmul(
                            pq[mi][:, ts * TC:(ts + 1) * TC],
                            w[:, co + mi * P:co + (mi + 1) * P],
                            in_tiles[k][:, ts * TC:(ts + 1) * TC],
                            start=(k == 0), stop=(k == n_in - 1))
            for mi in range(2):
                m = mgl * 2 + mi
                if post is not None:
                    post(m, pq[mi])
                    continue
                dst = out_tiles[m][:, 0:TL]
                src = pq[mi][0:P, 0:TL]
                if act == "sigmoid":
                    nc.scalar.activation(dst, src, AFT.Sigmoid)
                elif act == "sqrelu":
                    rl = xsc()
                    nc.scalar.activation(rl[:, 0:TL], src, AFT.Relu)
                    nc.vector.tensor_mul(dst, rl[:, 0:TL], rl[:, 0:TL])
                elif accum:
                    nc.vector.tensor_add(dst, dst, src)
                else:
                    nc.vector.tensor_copy(dst, src)

    # ================= phase 1: LN1 =================
    # stream xT once; cache a bf16 copy used for stats AND normalize.
    XCB = [bigt() for _ in range(NK)]
    for k in range(NK):
        xr = xsc()
        nc.sync.dma_start(xr[:], xT[k * P:(k + 1) * P, :])
        nc.vector.tensor_copy(XCB[k][:, 0:1 + TL], xr[:, 0:1 + TL])

    H1 = [bigt() for _ in range(NK)]
    ln_pass(lambda k, ts: XCB[k][:, 1 + ts * TC:1 + (ts + 1) * TC],
            lambda k: XCB[k][:, 0:1], LN1G, LN1B, H1, halo_mask=True)

    # ================= phase 2: K / V projections =================
    MQ = [bigt(cols=TL) for _ in range(NK)]
    for k in range(NK):
        mix(MQ[k], H1[k], MXK, MXK1, k)
    KT = [bigt(cols=TL) for _ in range(NK)]
    gemm_std(params["wk_t"], MQ, KT, NK)
    for k in range(NK):
        mix(MQ[k], H1[k], MXV, MXV1, k)
    # V transposed: VT[tslab (8 x 128 tokens)] as 2 tiles of [P, 1024] each.
    # wv_t blob cols ordered (cb, k, 512); stationary = MQ slab, moving = w.
    VT = [[bigt(cols=TL) for _ in range(2)] for _ in range(2 * 4)]
    for tg in range(2):       # t-slab groups of 4
        for cb in range(4):   # c_out banks of 512
            pv = [psa_() for _ in range(2)]
            for kq in range(4):
                w = wt_tile()
                nc.sync.dma_start(
                    w[:], params["wv_t"][:, (cb * NK + kq * 4) * TC:
                                         (cb * NK + kq * 4 + 4) * TC])
                for kk in range(4):
                    k = kq * 4 + kk
                    for ti in range(4):
                        tslab = tg * 4 + ti
                        nc.tensor.matmul(
                            pv[ti // 2][:, (ti % 2) * TC:(ti % 2 + 1) * TC],
                            MQ[k][:, tslab * P:(tslab + 1) * P],
                            w[:, kk * TC:(kk + 1) * TC],
                            start=(k == 0), stop=(k == NK - 1))
            for ti in range(4):
                nc.vector.tensor_copy(
                    VT[tg * 4 + ti][cb // 2][:, (cb % 2) * TC:(cb % 2 + 1) * TC],
                    pv[ti // 2][:, (ti % 2) * TC:(ti % 2 + 1) * TC])

    def vsl(i, j, h):
        """[P,S] value slice: chunk i, 128-token slab j, head h."""
        voff = h * S
        return VT[i * 4 + j][voff // TL][:, voff % TL:voff % TL + S]

    # ============ phase 3: state contributions + AllGather ============
    # K transposed per 128-token block with ONE full 128x128 transpose
    # covering both heads of the pair.
    CONTRIB0 = [pers.tile([P, S], f32, tag=f"c0_{p}", name=f"c0_{p}")
                for p in range(NP)]
    for p in range(NP):
        wkct = []
        for hh in range(2):
            h = 2 * p + hh
            t = sc((P, 4 * S), bf16)
            nc.scalar.activation(t[:], IOTAW[:], AFT.Exp,
                                 scale=LNW[:, h:h + 1])
            wkct.append(t)
        cts = []
        for i in range(NCH):
            ptrf = pst_((P, 4 * P), bf16)
            for j in range(4):
                nc.tensor.transpose(
                    ptrf[:, j * P:(j + 1) * P],
                    KT[p][:, i * TC + j * P:i * TC + (j + 1) * P],
                    IDENT[:])
            kw = sc((P, 4 * P), bf16)
            for j in range(4):
                for hh in range(2):
                    o = j * P + hh * S
                    nc.vector.tensor_mul(kw[:, o:o + S], ptrf[:, o:o + S],
                                         wkct[hh][:, j * S:(j + 1) * S])
            pst = pst_((P, S))
            for hh in range(2):
                h = 2 * p + hh
                pr = slice(hh * S, hh * S + S)
                for j in range(4):
                    nc.tensor.matmul(pst[pr, :], kw[:, j * P + hh * S:
                                                    j * P + hh * S + S],
                                     vsl(i, j, h),
                                     start=(j == 0), stop=(j == 3))
            if i == 0:
                nc.vector.tensor_copy(CONTRIB0[p][:], pst[:])
                cts.append(CONTRIB0[p])
            else:
                c1 = ssc()
                nc.vector.tensor_copy(c1[:], pst[:])
                cts.append(c1)
        so = ssc()
        nc.vector.scalar_tensor_tensor(so[:], cts[0][:], WSPP[:, p:p + 1],
                                       cts[1][:], AOT.mult, AOT.add)
        nc.sync.dma_start(sout_d[:, p * S:(p + 1) * S], so[:])
    nc.gpsimd.collective_compute("AllGather", AOT.bypass, replica_groups=GROUPS,
                                 ins=[sout_d.opt()], outs=[sgat_d.opt()])

    # ================= phase 4: R projection (overlaps AG) =================
    for k in range(NK):
        mix(MQ[k], H1[k], MXR, MXR1, k)
    H1 = None
    RT = [bigt(cols=TL) for _ in range(NK)]
    gemm_std(params["wr_t"], MQ, RT, NK)
    MQ = None

    # incoming state = smask * (rank0 shard of gather)
    SIN = pers.tile([P, NP * S], f32, tag="sin", name="sin")
    nc.sync.dma_start(SIN[:], sgat_d[0:P, :])
    nc.vector.tensor_scalar(SIN[:], SIN[:], SMB[:, 0:1], None, AOT.mult)

    # ================= phase 5: attention =================
    # Decay mask per head = column-shifted view of M_h[p,c]=w^(c-p) (c>=p)
    # plus diagonal-block D_h[p,q]=w^(q-p-1) (q>p) + u_h*I. Mask is zero
    # for t <= jP (except diag), so pa2/pout matmuls are restricted to
    # cols >= jP. Groupnorm stats are collected per (pair, chunk) into a
    # batched [64, TC] tile; one scalar chain per chunk; normalization
    # applied with gpsimd partition-broadcasts (no tensor-engine work).
    XA = [bigt(cols=TL) for _ in range(NK)]
    PSALL = {}

    def attn_pair(i, p, masks=None):
        h0, h1 = 2 * p, 2 * p + 1
        if masks is None:
            masks = build_masks(p)
        M_, D_ = masks
        wb = sc()
        nc.scalar.activation(wb[0:S, :], IOTA_T[0:S, :], AFT.Exp,
                             scale=LNW[0:S, h0:h0 + 1])
        nc.scalar.activation(wb[S:P, :], IOTA_T[S:P, :], AFT.Exp,
                             scale=LNW[S:P, h1:h1 + 1])
        st_mm = ssc((P, S), bf16)
        if i == 0:
            nc.vector.tensor_copy(st_mm[:], SIN[:, p * S:(p + 1) * S])
        else:
            stt = ssc()
            nc.vector.scalar_tensor_tensor(stt[:], SIN[:, p * S:(p + 1) * S],
                                           WSPP[:, p:p + 1], CONTRIB0[p][:],
                                           AOT.mult, AOT.add)
            nc.vector.tensor_copy(st_mm[:], stt[:])
        rtw = sc(dtype=bf16)
        nc.vector.tensor_mul(rtw[:], RT[p][:, i * TC:(i + 1) * TC], wb[:])
        pout = pst_((P, TC))

        def pout_mms(j, ast):
            for hh in range(2):
                h = 2 * p + hh
                pr = slice(hh * S, hh * S + S)
                nc.tensor.matmul(pout[pr, j * P:TC], vsl(i, j, h),
                                 ast[:, hh * TC + j * P:(hh + 1) * TC],
                                 start=False, stop=(j == 3))

        prev = None
        for j in range(4):
            pa2 = psa_()
            for hh in range(2):
                pr = slice(hh * S, hh * S + S)
                nc.tensor.matmul(
                    pa2[:, hh * TC + j * P:(hh + 1) * TC],
                    KT[p][pr, i * TC + j * P:i * TC + (j + 1) * P],
                    RT[p][pr, i * TC + j * P:(i + 1) * TC],
                    start=True, stop=True)
            if j == 0:
                # state term (start=True clears the pout bank); issued
                # after the first pa2 so the tensor queue isn't blocked
                # waiting on wb/rtw from the scalar/vector engines.
                for hh in range(2):
                    pr = slice(hh * S, hh * S + S)
                    nc.tensor.matmul(pout[pr, :], st_mm[pr, :], rtw[pr, :],
                                     start=True, stop=False)
            ast = sc((P, 2 * TC), bf16)
            for hh in range(2):
                o = hh * TC + j * P
                nc.vector.tensor_mul(ast[:, o:o + P], pa2[:, o:o + P],
                                     D_[hh][:])
                if j < 3:
                    nc.vector.tensor_mul(ast[:, o + P:(hh + 1) * TC],
                                         pa2[:, o + P:(hh + 1) * TC],
                                         M_[hh][:, P - 1:TC - j * P - 1])
            if prev is not None:
                pout_mms(*prev)
            prev = (j, ast)
        pout_mms(*prev)
        # raw attention out + gathered groupnorm stats (GSEL accumulate)
        nc.vector.tensor_copy(XA[p][:, i * TC:(i + 1) * TC], pout[:])
        sq = sc((P, TC), bf16)
        nc.vector.tensor_mul(sq[:], XA[p][:, i * TC:(i + 1) * TC],
                             XA[p][:, i * TC:(i + 1) * TC])
        if p == 0:
            PSALL[i] = psa_()
        sel = GSEL[:, 30 - 2 * p:62 - 2 * p]
        nc.tensor.matmul(PSALL[i][0:H, 0:TC], sel,
                         XA[p][:, i * TC:(i + 1) * TC],
                         start=(p == 0), stop=(p == NP - 1))
        nc.tensor.matmul(PSALL[i][0:H, TC:2 * TC], sel, sq[:],
                         start=(p == 0), stop=(p == NP - 1))

    def build_masks(p):
        M_ = []
        D_ = []
        for hh in range(2):
            h = 2 * p + hh
            m = mt_tile()
            nc.scalar.activation(m[:], IOTA_MM[:], AFT.Exp,
                                 scale=LNW[:, h:h + 1])
            dex = sc((P, P), bf16)
            nc.scalar.activation(dex[:], IOTA_DD[:], AFT.Exp,
                                 scale=LNW[:, h:h + 1])
            d = dt_tile()
            nc.vector.scalar_tensor_tensor(d[:], IDENT[:], UU[:, h:h + 1],
                                           dex[:], AOT.mult, AOT.add)
            M_.append(m)
            D_.append(d)
        return M_, D_

    def gn_chain(i):
        ps = PSALL.pop(i)
        m_ = sc((H, TC))
        nc.scalar.mul(m_[:], ps[0:H, 0:TC], 1.0 / (S * HS_DIV))
        q_ = sc((H, TC))
        nc.scalar.mul(q_[:], ps[0:H, TC:2 * TC],
                      1.0 / (S * HS_DIV * HS_DIV))
        msq = sc((H, TC)); nc.vector.tensor_mul(msq[:], m_[:], m_[:])
        var = sc((H, TC)); nc.vector.tensor_sub(var[:], q_[:], msq[:])
        lnv = sc((H, TC))
        nc.scalar.activation(lnv[:], var[:], AFT.Ln, bias=EPSB[0:H, 0:1])
        rs = sc((H, TC))
        nc.scalar.activation(rs[:], lnv[:], AFT.Exp, scale=-0.5)
        mrs = sc((H, TC))
        nc.vector.scalar_tensor_tensor(mrs[:], m_[:], -1.0, rs[:],
                                       AOT.mult, AOT.mult)
        rsh = sc((H, TC))
        nc.vector.tensor_scalar_mul(rsh[:], rs[:], 1.0 / HS_DIV)
        rsb = sc((H, TC), bf16)
        nc.vector.tensor_copy(rsb[:], rsh[:])
        mrb = sc((H, TC), bf16)
        nc.vector.tensor_copy(mrb[:], mrs[:])
        nc.sync.dma_start(rs_d[i][:], rsb[:])
        nc.sync.dma_start(mr_d[i][:], mrb[:])

    def gn_finish(i, p):
        # per-head broadcast of the groupnorm scale/shift rows via a DMA
        # bounce through DRAM (engines can't read non-32-aligned rows).
        brs = sc(dtype=bf16)
        bmrs = sc(dtype=bf16)
        for hh in range(2):
            r = 2 * p + hh
            pr = slice(hh * S, hh * S + S)
            nc.sync.dma_start(brs[pr, :],
                              rs_d[i][r:r + 1, :].partition_broadcast(S))
            nc.sync.dma_start(bmrs[pr, :],
                              mr_d[i][r:r + 1, :].partition_broadcast(S))
        xa = sc()
        nc.vector.tensor_mul(xa[:], XA[p][:, i * TC:(i + 1) * TC], brs[:])
        nc.vector.tensor_add(xa[:], xa[:], bmrs[:])
        nc.vector.tensor_scalar(XA[p][:, i * TC:(i + 1) * TC], xa[:],
                                LNXG[:, p:p + 1], LNXB[:, p:p + 1],
                                AOT.mult, AOT.add)

    for p in range(NP):
        attn_pair(0, p)
    mk0 = build_masks(0)
    mk1 = build_masks(1)
    attn_pair(1, 0, mk0)
    attn_pair(1, 1, mk1)
    gn_chain(0)
    for p in range(2, NP):
        attn_pair(1, p)
        gn_finish(0, p - 2)
    gn_finish(0, NP - 2)
    gn_finish(0, NP - 1)
    gn_chain(1)
    for p in range(NP):
        gn_finish(1, p)
    RT = KT = VT = None

    # ================= phase 6: Wo + residual, spill x' =================
    XP = [bigt(cols=TL) for _ in range(NK)]
    xr_tiles = {}

    def wo_pre(mgl):
        for m in (2 * mgl, 2 * mgl + 1):
            t = xsc()
            nc.sync.dma_start(t[:], xT[m * P:(m + 1) * P, :])
            xr_tiles[m] = t

    def wo_post(m, pq):
        xr = xr_tiles.pop(m)
        for ts in range(TS):
            c0, c1 = ts * TC, (ts + 1) * TC
            nc.vector.tensor_add(XP[m][:, c0:c1], pq[0:P, c0:c1],
                                 xr[:, 1 + c0:1 + c1])
            nc.sync.dma_start(xprime_d[m * P:(m + 1) * P, c0:c1],
                              XP[m][:, c0:c1])
        lc = sc((P, 1))
        nc.vector.tensor_copy(lc[:], XP[m][:, TL - 1:TL])
        nc.sync.dma_start(xcol_d[:, m:m + 1], lc[:])

    gemm_std(params["wo_t"], XA, XP, NK, post=wo_post, pre=wo_pre)
    XA = None
    nc.gpsimd.collective_compute("AllGather", AOT.bypass, replica_groups=GROUPS,
                                 ins=[xcol_d.opt()], outs=[xcgat_d.opt()])

    # ================= phase 7: LN2 + mixes =================
    XCH = const.tile([P, NK], f32, tag="xch")   # per-chunk halo cols
    nc.sync.dma_start(XCH[:], xcgat_d[0:P, :])
    XCHB = const.tile([P, NK], bf16, tag="xchb")
    nc.vector.tensor_copy(XCHB[:], XCH[:])

    H2 = [bigt() for _ in range(NK)]
    ln_pass(lambda k, ts: XP[k][:, ts * TC:(ts + 1) * TC],
            lambda k: XCHB[:, k:k + 1], LN2G, LN2B, H2, halo_mask=True)
    XP = None
    MFK = [bigt(cols=TL) for _ in range(NK)]
    MFR = [bigt(cols=TL) for _ in range(NK)]
    for k in range(NK):
        mix(MFK[k], H2[k], FMK, FMK1, k)
        mix(MFR[k], H2[k], FMR, FMR1, k)
    H2 = None

    # ================= phase 8: gate = sigmoid(mfr @ wfr) =================
    GT = [bigt(cols=TL) for _ in range(NK)]
    gemm_std(params["wfr_t"], MFR, GT, NK, act="sigmoid")
    MFR = None

    # ========== phase 9: FFN quarters: kf=relu^2(mfk@wfk); kv+=wfv^T@kf ==========
    # last quarter's Wfv finalize writes y = x' + gate*kv directly.
    KV = [bigt(cols=TL) for _ in range(NK)]
    xp_tiles = {}

    def y_pre(mgl):
        for m in (2 * mgl, 2 * mgl + 1):
            t = sc((P, TL), bf16)
            nc.sync.dma_start(t[:], xprime_d[m * P:(m + 1) * P, :])
            xp_tiles[m] = t

    def y_post(m, pq):
        xp = xp_tiles.pop(m)
        for ts in range(TS):
            c0, c1 = ts * TC, (ts + 1) * TC
            kvf = sc()
            nc.vector.tensor_add(kvf[:], KV[m][:, c0:c1], pq[0:P, c0:c1])
            gk = sc()
            nc.vector.tensor_mul(gk[:], GT[m][:, c0:c1], kvf[:])
            yo = sc()
            nc.vector.tensor_add(yo[:], xp[:, c0:c1], gk[:])
            nc.sync.dma_start(yT[m * P:(m + 1) * P, c0:c1], yo[:])

    for q in range(NQ):
        KF = [bigt(cols=TL) for _ in range(JQ)]
        gemm_std(params["wfk_t"], MFK, KF, JQ, act="sqrelu",
                 col_base=q * (JQ // 2) * NK * 2 * P)
        # kv partial: contract the quarter's 14 j-chunks
        if q < NQ - 1:
            gemm_std(params["wfv_t"], KF, KV, NK, accum=(q > 0), n_in=JQ, G=7,
                     col_base=q * (NK // 2) * JQ * 2 * P)
        else:
            gemm_std(params["wfv_t"], KF, KV, NK, n_in=JQ, G=7,
                     col_base=q * (NK // 2) * JQ * 2 * P,
                     post=y_post, pre=y_pre)
        KF = None
    MFK = None

    for c in reversed(ctxs):
        c.__exit__(None, None, None)


# ----------------------------------------------------------------------
# Host-side sharding / gather
# ----------------------------------------------------------------------
import ml_dtypes

_NC_CACHE = {}


def _vec_pk(v, nk=NK):
    return np.ascontiguousarray(np.asarray(v).reshape(nk, P).T.astype(np.float32))


def _make_in_maps(inputs):
    x = np.asarray(inputs["x"], np.float32)
    bf = ml_dtypes.bfloat16
    td = np.asarray(inputs["time_decay"], np.float64)
    w = np.exp(-np.exp(td))                      # [H]
    ws = w ** TC
    wspp = np.zeros((P, NP), np.float32)
    for p in range(NP):
        wspp[0:S, p] = ws[2 * p]
        wspp[S:P, p] = ws[2 * p + 1]
    def _tile_mk(W):
        """[n_in*128, n_mg*256] -> [128, n_mg*n_in*256], cols (mg, k, c)."""
        n_in = W.shape[0] // P
        n_mg = W.shape[1] // (2 * P)
        return np.ascontiguousarray(
            W.reshape(n_in, P, n_mg, 2 * P).transpose(1, 2, 0, 3)
             .reshape(P, -1).astype(bf))

    wcache = {}
    for nm, key in [("wr_t", "Wr"), ("wk_t", "Wk"), ("wo_t", "Wo"),
                    ("wfr_t", "Wfr"), ("wfk_t", "Wfk")]:
        wcache[nm] = _tile_mk(np.asarray(inputs[key], np.float32))
    Wfv = np.asarray(inputs["Wfv"], np.float32)
    wcache["wfv_t"] = np.ascontiguousarray(np.concatenate(
        [_tile_mk(Wfv[q * JQ * P:(q + 1) * JQ * P, :]) for q in range(NQ)],
        axis=1))
    Wv = np.asarray(inputs["Wv"], np.float32)
    wcache["wv_t"] = np.ascontiguousarray(
        Wv.reshape(NK, P, 4, TC).transpose(1, 2, 0, 3).reshape(P, -1).astype(bf))
    maps = []
    for c in range(8):
        b, half = c // 2, c % 2
        t0 = half * TL
        xh = np.zeros((C, 1 + TL), np.float32)
        xh[:, 1:] = x[b, t0:t0 + TL, :].T
        if half == 1:
            xh[:, 0] = x[b, t0 - 1, :]
        maps.append({
            "xT": np.ascontiguousarray(xh),
            **wcache,
            "wspp": wspp,
            "smask": np.full((1, 1), float(half), np.float32),
            "ln1g": _vec_pk(inputs["ln1_g"]), "ln1b": _vec_pk(inputs["ln1_b"]),
            "ln2g": _vec_pk(inputs["ln2_g"]), "ln2b": _vec_pk(inputs["ln2_b"]),
            "mxk": _vec_pk(inputs["att_mix_k"]), "mxv": _vec_pk(inputs["att_mix_v"]),
            "mxr": _vec_pk(inputs["att_mix_r"]),
            "fmk": _vec_pk(inputs["ffn_mix_k"]), "fmr": _vec_pk(inputs["ffn_mix_r"]),
            "lnxg": _vec_pk(inputs["lnx_g"], NP),
            "lnxb": _vec_pk(inputs["lnx_b"], NP),
            "tdv": np.ascontiguousarray(np.asarray(inputs["time_decay"],
                                                   np.float32)[None, :]),
            "uv": np.ascontiguousarray(np.asarray(inputs["time_faaaa"],
                                                  np.float32)[None, :]),
        })
    return maps


def run_on_hw(inputs, trace=False):
    from concourse.bass_utils import run_bass_kernel_spmd
    if "nc" not in _NC_CACHE:
        _NC_CACHE["nc"] = build_nc()
    nc = _NC_CACHE["nc"]
    maps = _make_in_maps(inputs)
    res = run_bass_kernel_spmd(nc, maps, core_ids=list(range(8)), trace=trace)
    B = 4
    out = np.zeros((B, 2 * TL, C), np.float32)
    for c in range(8):
        b, half = c // 2, c % 2
        out[b, half * TL:(half + 1) * TL, :] = res.results[c]["yT"].T
    return out, res


def kernel(**inputs) -> np.ndarray:
    out, _ = run_on_hw(inputs, trace=False)
    return out
